# revision 37
# baseline (speedup 1.0000x reference)
"""Causal self-attention (B=4, T=2048, C=2048, H=16, D=128) on 8 TRN2 NeuronCores.

Sharding: 8 shards = (batch b in 0..3) x (head-group hg in {0,1}, 8 heads each).
Each core computes qkv for its (batch, 8 heads), causal attention, and a
partial output projection y_hg @ w_proj[hg-rows]; the host sums the two
partials per batch and adds b_proj.

Device compute is bf16 (fp32 PSUM accumulation); inputs are cast/sliced/
pre-transposed on the host so the device never transposes anything:
  - x is shipped transposed (xT, [C, T] tiled as [128, 16, 2048]).
  - qT/kT computed as w.T @ xT (output [head-dim, T]) -> directly usable as
    matmul operands for S^T = k.T-tile @ qT.
  - softmax runs on S^T (keys on partitions): exp on ScalarE (no max
    subtraction needed; logits are O(1)), causal masking via precomputed 0/1
    mask tiles, denominators via all-ones-matrix matmuls (column sums arrive
    pre-broadcast across partitions), reciprocal_approx_fast + multiply for
    the normalization, decoupled from the PE pipeline.
  - v computed in natural [T, head-dim] layout (lhsT = xT tiles) so the AV
    matmul yT += v-block.T @ P^T needs no transpose either.
  - out = y @ w_proj with lhsT = yT tiles, rhs = w_proj slices.
1/sqrt(D) is folded into w_q (and b_q) on the host.

Scheduling: the PE executes in emission order, so exp-independent projection
matmuls (the next head's q/k projection, or the output projection for the
last head) are woven between attention pairs in 2-matmul steps to fill the
exp-latency stalls. PSUM: 3 shared 2-bank strips + 1 y-accumulator bank +
1 denominator bank = 8 banks.
"""

import math
from contextlib import ExitStack

import ml_dtypes
import numpy as np

import concourse.bass as bass  # noqa: F401  (bass types used via tile/bacc)
import concourse.tile as tile
from concourse import bacc, mybir
from concourse.alu_op_type import AluOpType
from concourse.bass_utils import run_bass_kernel_spmd

P = 128
B, T, C, H, D = 4, 2048, 2048, 16, 128
HG = 2              # head groups (tensor-parallel factor); B * HG = 8 cores
HL = H // HG        # heads per core
KT = C // P         # 16 contraction tiles
TCH = T // 512      # 4 query chunks of 512
BF16 = mybir.dt.bfloat16
F32 = mybir.dt.float32
EXP = mybir.ActivationFunctionType.Exp

_NC_CACHE = []
LAST_RESULTS = None  # BassKernelResults of the most recent kernel() call


def build_nc():
    nc = bacc.Bacc("TRN2", target_bir_lowering=False, debug=False, num_devices=8)

    xt_d = nc.dram_tensor("xt", [P, KT, T], BF16, kind="ExternalInput")
    wq_d = nc.dram_tensor("wq", [HL, P, KT, P], BF16, kind="ExternalInput")
    wk_d = nc.dram_tensor("wk", [HL, P, KT, P], BF16, kind="ExternalInput")
    wv_d = nc.dram_tensor("wv", [P, KT, HL * D], BF16, kind="ExternalInput")
    wp_d = nc.dram_tensor("wp", [P, HL, C], BF16, kind="ExternalInput")
    bq_d = nc.dram_tensor("bq", [P, HL], F32, kind="ExternalInput")
    bk_d = nc.dram_tensor("bk", [P, HL], F32, kind="ExternalInput")
    bv_d = nc.dram_tensor("bv", [P, HL * D], F32, kind="ExternalInput")
    mk_d = nc.dram_tensor("masks", [P, 2, 1024], BF16, kind="ExternalInput")
    out_d = nc.dram_tensor("out", [T, C], F32, kind="ExternalOutput")

    with nc.allow_low_precision("bf16 attention kernel"), \
         tile.TileContext(nc) as tc, ExitStack() as ctx:
        consts = ctx.enter_context(tc.tile_pool(name="consts", bufs=1))
        big = ctx.enter_context(tc.tile_pool(name="big", bufs=1))      # xt -> wp
        vpool = ctx.enter_context(tc.tile_pool(name="v", bufs=1))
        mid = ctx.enter_context(tc.tile_pool(name="mid", bufs=1))      # wv -> yt
        qk = ctx.enter_context(tc.tile_pool(name="qk", bufs=2))
        wcol = ctx.enter_context(tc.tile_pool(name="wcol", bufs=3))
        ppool = ctx.enter_context(tc.tile_pool(name="p", bufs=8))
        spool = ctx.enter_context(tc.tile_pool(name="small", bufs=3))
        opool = ctx.enter_context(tc.tile_pool(name="o", bufs=2))
        # PSUM: 3 shared 2-bank strips + ps_y/ps_b slot + ps_d slot = 8 banks
        mm = ctx.enter_context(tc.tile_pool(name="mm", bufs=3, space="PSUM"))
        psy = ctx.enter_context(tc.tile_pool(name="psy", bufs=1, space="PSUM"))
        psd = ctx.enter_context(tc.tile_pool(name="psd", bufs=1, space="PSUM"))

        # tiny per-partition biases (sync queue, ahead of the xT strips)
        bq_sb = consts.tile([P, HL], F32)
        nc.sync.dma_start(bq_sb[:], bq_d[:])
        bk_sb = consts.tile([P, HL], F32)
        nc.sync.dma_start(bk_sb[:], bk_d[:])
        ones_mat = consts.tile([P, P], BF16)
        nc.vector.memset(ones_mat[:], 1.0)

        # xT strips on the sync queue; head-0 q/k consumes them as they land
        xt = big.tile([P, KT, T], BF16, tag="big")
        for kt in range(KT):
            nc.sync.dma_start(xt[:, kt, :], xt_d[:, kt, :])
        wv = mid.tile([P, KT, HL * D], BF16, tag="mid")
        nc.sync.dma_start(wv[:], wv_d[:])

        qk_tiles = {}

        class QKFiller:
            """Emits head h's q/k projection in 2-matmul steps (one kt each),
            so steps can be woven between attention pairs of head h-1."""

            def __init__(self, h):
                self.h, self.unit, self.kt, self.ps = h, 0, 0, None

            def step(self):
                h, unit = self.h, self.unit
                if unit >= 4:
                    return False
                which = "q" if unit < 2 else "k"
                if self.kt == 0:
                    if unit in (0, 2):
                        dst = qk.tile([P, T], BF16,
                                      tag="qh" if unit == 0 else "kh",
                                      name=f"{which}h{h}")
                        qk_tiles[(h, which)] = dst
                        wc_t = wcol.tile([P, KT, P], BF16, tag="wcol",
                                         name=f"wc_{which}{h}")
                        qk_tiles[(h, which + "w")] = wc_t
                        nc.gpsimd.dma_start(wc_t[:], (wq_d if unit == 0 else wk_d)[h])
                    self.ps = mm.tile([P, 1024], F32, tag="mm", name=f"qkps{h}")
                wc_t = qk_tiles[(h, which + "w")]
                pair = unit % 2
                for t2 in range(2):
                    tci = pair * 2 + t2
                    nc.tensor.matmul(
                        self.ps[:, t2 * 512:(t2 + 1) * 512],
                        wc_t[:, self.kt, :],
                        xt[:, self.kt, tci * 512:(tci + 1) * 512],
                        start=(self.kt == 0), stop=(self.kt == KT - 1),
                    )
                self.kt += 1
                if self.kt == KT:
                    b_sb = bq_sb if unit < 2 else bk_sb
                    nc.vector.tensor_tensor(
                        qk_tiles[(h, which)][:, pair * 1024:(pair + 1) * 1024],
                        self.ps[:],
                        b_sb[:, h:h + 1].to_broadcast((P, 1024)),
                        AluOpType.add,
                    )
                    self.kt, self.ps = 0, None
                    self.unit += 1
                return True

            def drain(self):
                while self.step():
                    pass

        class OutFiller:
            """Emits output-projection (tt, pair) units in 2-matmul steps."""

            def __init__(self, wp):
                self.wp, self.q, self.cur, self.hh, self.ps = wp, [], None, 0, None

            def add(self, tt):
                self.q += [(tt, 0), (tt, 1)]

            def step(self):
                if self.cur is None:
                    if not self.q:
                        return False
                    self.cur = self.q.pop(0)
                    self.hh = 0
                    self.ps = mm.tile([P, 1024], F32, tag="mm", name="ops")
                tt, pair = self.cur
                for c2 in range(2):
                    nc.tensor.matmul(
                        self.ps[:, c2 * 512:(c2 + 1) * 512],
                        yt[:, self.hh, tt * P:(tt + 1) * P],
                        self.wp[:, self.hh, (pair * 2 + c2) * 512:
                                (pair * 2 + c2 + 1) * 512],
                        start=(self.hh == 0), stop=(self.hh == HL - 1),
                    )
                self.hh += 1
                if self.hh == HL:
                    ot = opool.tile([P, 1024], F32)
                    nc.vector.tensor_copy(ot[:], self.ps[:])
                    nc.sync.dma_start(
                        out_d[tt * P:(tt + 1) * P,
                              pair * 1024:(pair + 1) * 1024], ot[:])
                    self.cur, self.ps = None, None
                return True

            def drain(self):
                while self.step():
                    pass

        # head 0's q/k: consumes xT strips as they arrive
        QKFiller(0).drain()

        # masks / bv ride the sync queue behind wv (needed only once
        # attention starts); the gpsimd queue stays clear for q/k weights
        mk_sb = consts.tile([P, 2, 1024], BF16)
        nc.sync.dma_start(mk_sb[:], mk_d[:])
        bv_sb = consts.tile([P, HL * D], F32)
        nc.sync.dma_start(bv_sb[:], bv_d[:])

        # stage B: v = x @ w_v, natural layout v_sb[p=T within tt, tt, hl*D]
        v_sb = vpool.tile([P, KT, HL * D], BF16)
        for tt in range(KT):
            ps = mm.tile([P, 1024], F32, tag="mm")
            for kt in range(KT):
                for wc in range(2):
                    nc.tensor.matmul(
                        ps[:, wc * 512:(wc + 1) * 512],
                        xt[:, kt, tt * P:(tt + 1) * P],
                        wv[:, kt, wc * 512:(wc + 1) * 512],
                        start=(kt == 0), stop=(kt == KT - 1),
                    )
            nc.vector.tensor_tensor(
                v_sb[:, tt, :], ps[:], bv_sb[:], AluOpType.add,
            )

        yt = mid.tile([P, HL, T], BF16, tag="mid")  # waits until wv is dead

        wp = None
        for h in range(HL):
            qh = qk_tiles[(h, "q")]
            kh = qk_tiles[(h, "k")]
            if h + 1 < HL:
                filler = QKFiller(h + 1)
                steps_total = 4 * KT
            else:
                # xt is dead after head 7's q/k; stream w_proj in and emit the
                # output projection as this head's attention chunks complete
                wp = big.tile([P, HL, C], BF16, tag="big")
                nc.sync.dma_start(wp[:], wp_d[:])
                filler = OutFiller(wp)
                steps_total = 0  # supply added per chunk
            acc = 0.0
            pairs_left = sum(2 * (c + 1) for c in range(TCH))

            for c in range(TCH):
                jmax = 4 * c + 3
                npair = (jmax + 1) // 2
                ps_y = psy.tile([P, 512], F32, tag="y")
                ps_d = psd.tile([P, 512], F32)
                pend = None   # pending pair-sum for the denominator tree
                pend4 = None  # pending 4-sum (tile, contains_jp0)
                for jp in range(npair):
                    diag = 2 * jp >= 4 * c
                    # in diagonal blocks, columns i < 128p of j-tile offset p
                    # are fully causal-masked: narrow S/exp/mask/AV/ones to
                    # the live range instead of computing and zeroing them
                    skips = []
                    ps_s = mm.tile([P, 1024], F32, tag="mm")
                    for half in range(2):
                        jt = 2 * jp + half
                        sk = 128 * (jt - 4 * c) if diag and jt > 4 * c else 0
                        skips.append(sk)
                        nc.tensor.matmul(
                            ps_s[:, half * 512 + sk:(half + 1) * 512],
                            kh[:, jt * P:(jt + 1) * P],
                            qh[:, c * 512 + sk:(c + 1) * 512],
                            start=True, stop=True,
                        )
                    pt = ppool.tile([P, 1024], BF16)
                    if not diag:
                        nc.scalar.activation(pt[:], ps_s[:], EXP)
                    else:
                        p2 = jp - 2 * c
                        for half in range(2):
                            sk = skips[half]
                            sl = slice(half * 512 + sk, (half + 1) * 512)
                            nc.scalar.activation(pt[:, sl], ps_s[:, sl], EXP)
                            nc.vector.tensor_tensor(
                                pt[:, sl], pt[:, sl], mk_sb[:, p2, sl],
                                AluOpType.mult,
                            )
                    for half in range(2):
                        jt = 2 * jp + half
                        sk = skips[half]
                        nc.tensor.matmul(
                            ps_y[:, sk:512], v_sb[:, jt, h * D:(h + 1) * D],
                            pt[:, half * 512 + sk:(half + 1) * 512],
                            start=(jt == 0), stop=(jt == jmax),
                        )
                    if not diag:
                        # column-sum of (p0 + p1) == sum of both column-sums:
                        # pre-sum the pair on DVE; combine two pair-sums into
                        # one ones-matmul (4-way tree, non-diag count is even)
                        pts = spool.tile([P, 512], BF16, tag="pts")
                        nc.vector.tensor_tensor(
                            pts[:], pt[:, 0:512], pt[:, 512:1024], AluOpType.add
                        )
                        if pend is None:
                            pend = (pts, jp)
                        else:
                            pts2 = spool.tile([P, 512], BF16, tag="pts2")
                            nc.vector.tensor_tensor(
                                pts2[:], pend[0][:], pts[:], AluOpType.add
                            )
                            if pend4 is None:
                                pend4 = (pts2, pend[1] == 0)
                            else:
                                pts3 = spool.tile([P, 512], BF16, tag="pts2")
                                nc.vector.tensor_tensor(
                                    pts3[:], pend4[0][:], pts2[:], AluOpType.add
                                )
                                nc.tensor.matmul(
                                    ps_d[:], ones_mat[:], pts3[:],
                                    start=pend4[1], stop=False,
                                )
                                pend4 = None
                            pend = None
                    else:
                        # flush any pending 4-sum BEFORE diagonal ones-matmuls
                        # so the group's start=True matmul executes first
                        if pend4 is not None:
                            nc.tensor.matmul(
                                ps_d[:], ones_mat[:], pend4[0][:],
                                start=pend4[1], stop=False,
                            )
                            pend4 = None
                        for half in range(2):
                            jt = 2 * jp + half
                            sk = skips[half]
                            nc.tensor.matmul(
                                ps_d[:, sk:512], ones_mat[:],
                                pt[:, half * 512 + sk:(half + 1) * 512],
                                start=(jt == 0), stop=(jt == jmax),
                            )
                    # weave in exp-independent projection matmuls
                    if h + 1 < HL:
                        rate = 2.0 if h + 1 == HL - 1 else 1.0
                        acc += rate * steps_total / pairs_left
                        while acc >= 1.0 and filler.step():
                            acc -= 1.0
                    else:
                        for _ in range(3):
                            filler.step()
                assert pend is None and pend4 is None  # flushed at first diag
                # normalization, decoupled from the PE pipeline
                yu = spool.tile([P, 512], F32, tag="yu")
                nc.vector.tensor_copy(yu[:], ps_y[:])
                rc = spool.tile([P, 512], F32, tag="rc")
                nc.vector.reciprocal_approx_fast(rc[:], ps_d[:])
                nc.vector.tensor_tensor(
                    yt[:, h, c * 512:(c + 1) * 512], yu[:], rc[:], AluOpType.mult
                )
                if h + 1 == HL:
                    # rows 4c..4c+3 of the output are now fully determined
                    for tt in range(4 * c, 4 * c + 4):
                        filler.add(tt)
            filler.drain()

    nc.compile()
    return nc


def get_nc():
    if not _NC_CACHE:
        _NC_CACHE.append(build_nc())
    return _NC_CACHE[0]


def _bf(a):
    return np.ascontiguousarray(a).astype(ml_dtypes.bfloat16)


def _shard_inputs(x, w_attn, b_attn, w_proj):
    """Build the 8 per-core in_maps."""
    scale = 1.0 / math.sqrt(D)
    w_q, w_k, w_v = w_attn[:, :C], w_attn[:, C:2 * C], w_attn[:, 2 * C:]
    b_q, b_k, b_v = b_attn[:C], b_attn[C:2 * C], b_attn[2 * C:]

    # causal masks for the 4 j-tile positions within a 512 diagonal block,
    # paired as [2, 128, 1024]: pair 0 = (jt offset 0, 1), pair 1 = (2, 3)
    j = np.arange(P)[:, None]
    i = np.arange(512)[None, :]
    m4 = [(j + P * p <= i).astype(np.float32) for p in range(4)]
    masks = np.stack([np.concatenate([m4[0], m4[1]], axis=1),
                      np.concatenate([m4[2], m4[3]], axis=1)])
    masks_bf = _bf(masks.transpose(1, 0, 2))  # [128, 2, 1024]

    per_hg = {}
    for hg in range(HG):
        s = slice(hg * HL * D, (hg + 1) * HL * D)
        wq = _bf((w_q[:, s] * scale).reshape(KT, P, HL, D).transpose(2, 1, 0, 3))
        wk = _bf(w_k[:, s].reshape(KT, P, HL, D).transpose(2, 1, 0, 3))
        wv = _bf(w_v[:, s].reshape(KT, P, HL * D).transpose(1, 0, 2))
        wp = _bf(w_proj[s, :].reshape(HL, P, C).transpose(1, 0, 2))
        bq = np.ascontiguousarray(
            (b_q[s] * scale).reshape(HL, P).T).astype(np.float32)
        bk = np.ascontiguousarray(b_k[s].reshape(HL, P).T).astype(np.float32)
        bv = np.ascontiguousarray(
            np.broadcast_to(b_v[s], (P, HL * D))).astype(np.float32)
        per_hg[hg] = dict(wq=wq, wk=wk, wv=wv, wp=wp, bq=bq, bk=bk, bv=bv)

    in_maps = []
    for core in range(8):
        b, hg = core // HG, core % HG
        xt = _bf(x[b].T.reshape(KT, P, T).transpose(1, 0, 2))
        in_maps.append({"xt": xt, "masks": masks_bf, **per_hg[hg]})
    return in_maps


def kernel(x, w_attn, b_attn, w_proj, b_proj):
    global LAST_RESULTS
    x = np.asarray(x, dtype=np.float32)
    w_attn = np.asarray(w_attn, dtype=np.float32)
    b_attn = np.asarray(b_attn, dtype=np.float32)
    w_proj = np.asarray(w_proj, dtype=np.float32)
    b_proj = np.asarray(b_proj, dtype=np.float32)

    nc = get_nc()
    in_maps = _shard_inputs(x, w_attn, b_attn, w_proj)
    res = run_bass_kernel_spmd(nc, in_maps, core_ids=list(range(8)))
    LAST_RESULTS = res

    out = np.empty((B, T, C), dtype=np.float32)
    for b in range(B):
        out[b] = res.results[HG * b]["out"] + res.results[HG * b + 1]["out"]
        out[b] += b_proj[None, :]
    return out


# revision 39
# speedup vs baseline: 1.0079x; 1.0079x over previous
"""Causal self-attention (B=4, T=2048, C=2048, H=16, D=128) on 8 TRN2 NeuronCores.

Sharding: 8 shards = (batch b in 0..3) x (head-group hg in {0,1}, 8 heads each).
Each core computes qkv for its (batch, 8 heads), causal attention, and a
partial output projection y_hg @ w_proj[hg-rows]; the host sums the two
partials per batch and adds b_proj.

Device compute is bf16 (fp32 PSUM accumulation); inputs are cast/sliced/
pre-transposed on the host so the device never transposes anything:
  - x is shipped transposed (xT, [C, T] tiled as [128, 16, 2048]).
  - qT/kT computed as w.T @ xT (output [head-dim, T]) -> directly usable as
    matmul operands for S^T = k.T-tile @ qT.
  - softmax runs on S^T (keys on partitions): exp on ScalarE (no max
    subtraction needed; logits are O(1)), causal masking via precomputed 0/1
    mask tiles, denominators via all-ones-matrix matmuls (column sums arrive
    pre-broadcast across partitions), reciprocal_approx_fast + multiply for
    the normalization, decoupled from the PE pipeline.
  - v computed in natural [T, head-dim] layout (lhsT = xT tiles) so the AV
    matmul yT += v-block.T @ P^T needs no transpose either.
  - out = y @ w_proj with lhsT = yT tiles, rhs = w_proj slices.
1/sqrt(D) is folded into w_q (and b_q) on the host.

Scheduling: the PE executes in emission order, so exp-independent projection
matmuls (the next head's q/k projection, or the output projection for the
last head) are woven between attention pairs in 2-matmul steps to fill the
exp-latency stalls. PSUM: 3 shared 2-bank strips + 1 y-accumulator bank +
1 denominator bank = 8 banks.
"""

import math
from contextlib import ExitStack

import ml_dtypes
import numpy as np

import concourse.bass as bass  # noqa: F401  (bass types used via tile/bacc)
import concourse.tile as tile
from concourse import bacc, mybir
from concourse.alu_op_type import AluOpType
from concourse.bass_utils import run_bass_kernel_spmd

P = 128
B, T, C, H, D = 4, 2048, 2048, 16, 128
HG = 2              # head groups (tensor-parallel factor); B * HG = 8 cores
HL = H // HG        # heads per core
KT = C // P         # 16 contraction tiles
TCH = T // 512      # 4 query chunks of 512
BF16 = mybir.dt.bfloat16
F32 = mybir.dt.float32
EXP = mybir.ActivationFunctionType.Exp

_NC_CACHE = []
LAST_RESULTS = None  # BassKernelResults of the most recent kernel() call


def build_nc():
    nc = bacc.Bacc("TRN2", target_bir_lowering=False, debug=False, num_devices=8)

    xt_d = nc.dram_tensor("xt", [P, KT, T], BF16, kind="ExternalInput")
    wq_d = nc.dram_tensor("wq", [HL, P, KT, P], BF16, kind="ExternalInput")
    wk_d = nc.dram_tensor("wk", [HL, P, KT, P], BF16, kind="ExternalInput")
    wv_d = nc.dram_tensor("wv", [P, KT, HL * D], BF16, kind="ExternalInput")
    wp_d = nc.dram_tensor("wp", [P, HL, C], BF16, kind="ExternalInput")
    bq_d = nc.dram_tensor("bq", [P, HL], F32, kind="ExternalInput")
    bk_d = nc.dram_tensor("bk", [P, HL], F32, kind="ExternalInput")
    bv_d = nc.dram_tensor("bv", [P, HL * D], F32, kind="ExternalInput")
    mk_d = nc.dram_tensor("masks", [P, 2, 1024], BF16, kind="ExternalInput")
    out_d = nc.dram_tensor("out", [T, C], F32, kind="ExternalOutput")

    with nc.allow_low_precision("bf16 attention kernel"), \
         tile.TileContext(nc) as tc, ExitStack() as ctx:
        consts = ctx.enter_context(tc.tile_pool(name="consts", bufs=1))
        big = ctx.enter_context(tc.tile_pool(name="big", bufs=1))      # xt -> wp
        vpool = ctx.enter_context(tc.tile_pool(name="v", bufs=1))
        mid = ctx.enter_context(tc.tile_pool(name="mid", bufs=1))      # wv -> yt
        qk = ctx.enter_context(tc.tile_pool(name="qk", bufs=2))
        wcol = ctx.enter_context(tc.tile_pool(name="wcol", bufs=3))
        ppool = ctx.enter_context(tc.tile_pool(name="p", bufs=8))
        spool = ctx.enter_context(tc.tile_pool(name="small", bufs=3))
        opool = ctx.enter_context(tc.tile_pool(name="o", bufs=2))
        # PSUM: 3 shared 2-bank strips + ps_y/ps_b slot + ps_d slot = 8 banks
        mm = ctx.enter_context(tc.tile_pool(name="mm", bufs=3, space="PSUM"))
        psy = ctx.enter_context(tc.tile_pool(name="psy", bufs=1, space="PSUM"))
        psd = ctx.enter_context(tc.tile_pool(name="psd", bufs=1, space="PSUM"))

        # tiny per-partition biases (sync queue, ahead of the xT strips)
        bq_sb = consts.tile([P, HL], F32)
        nc.sync.dma_start(bq_sb[:], bq_d[:])
        bk_sb = consts.tile([P, HL], F32)
        nc.sync.dma_start(bk_sb[:], bk_d[:])
        ones_mat = consts.tile([P, P], BF16)
        nc.vector.memset(ones_mat[:], 1.0)

        # xT strips on the sync queue; head-0 q/k consumes them as they land
        xt = big.tile([P, KT, T], BF16, tag="big")
        for kt in range(KT):
            nc.sync.dma_start(xt[:, kt, :], xt_d[:, kt, :])
        wv = mid.tile([P, KT, HL * D], BF16, tag="mid")
        nc.sync.dma_start(wv[:], wv_d[:])

        qk_tiles = {}

        class QKFiller:
            """Emits head h's q/k projection in 2-matmul steps (one kt each),
            so steps can be woven between attention pairs of head h-1."""

            def __init__(self, h):
                self.h, self.unit, self.kt, self.ps = h, 0, 0, None

            def step(self):
                h, unit = self.h, self.unit
                if unit >= 4:
                    return False
                which = "q" if unit < 2 else "k"
                if self.kt == 0:
                    if unit in (0, 2):
                        dst = qk.tile([P, T], BF16,
                                      tag="qh" if unit == 0 else "kh",
                                      name=f"{which}h{h}")
                        qk_tiles[(h, which)] = dst
                        wc_t = wcol.tile([P, KT, P], BF16, tag="wcol",
                                         name=f"wc_{which}{h}")
                        qk_tiles[(h, which + "w")] = wc_t
                        nc.gpsimd.dma_start(wc_t[:], (wq_d if unit == 0 else wk_d)[h])
                    self.ps = mm.tile([P, 1024], F32, tag="mm", name=f"qkps{h}")
                wc_t = qk_tiles[(h, which + "w")]
                pair = unit % 2
                for t2 in range(2):
                    tci = pair * 2 + t2
                    nc.tensor.matmul(
                        self.ps[:, t2 * 512:(t2 + 1) * 512],
                        wc_t[:, self.kt, :],
                        xt[:, self.kt, tci * 512:(tci + 1) * 512],
                        start=(self.kt == 0), stop=(self.kt == KT - 1),
                    )
                self.kt += 1
                if self.kt == KT:
                    b_sb = bq_sb if unit < 2 else bk_sb
                    nc.vector.tensor_tensor(
                        qk_tiles[(h, which)][:, pair * 1024:(pair + 1) * 1024],
                        self.ps[:],
                        b_sb[:, h:h + 1].to_broadcast((P, 1024)),
                        AluOpType.add,
                    )
                    self.kt, self.ps = 0, None
                    self.unit += 1
                return True

            def drain(self):
                while self.step():
                    pass

        class OutFiller:
            """Emits output-projection (tt, pair) units in 2-matmul steps."""

            def __init__(self, wp):
                self.wp, self.q, self.cur, self.hh, self.ps = wp, [], None, 0, None

            def add(self, tt):
                self.q += [(tt, 0), (tt, 1)]

            def step(self):
                if self.cur is None:
                    if not self.q:
                        return False
                    self.cur = self.q.pop(0)
                    self.hh = 0
                    self.ps = mm.tile([P, 1024], F32, tag="mm", name="ops")
                tt, pair = self.cur
                for c2 in range(2):
                    nc.tensor.matmul(
                        self.ps[:, c2 * 512:(c2 + 1) * 512],
                        yt[:, self.hh, tt * P:(tt + 1) * P],
                        self.wp[:, self.hh, (pair * 2 + c2) * 512:
                                (pair * 2 + c2 + 1) * 512],
                        start=(self.hh == 0), stop=(self.hh == HL - 1),
                    )
                self.hh += 1
                if self.hh == HL:
                    ot = opool.tile([P, 1024], F32)
                    nc.vector.tensor_copy(ot[:], self.ps[:])
                    nc.sync.dma_start(
                        out_d[tt * P:(tt + 1) * P,
                              pair * 1024:(pair + 1) * 1024], ot[:])
                    self.cur, self.ps = None, None
                return True

            def drain(self):
                while self.step():
                    pass

        # head 0's q/k: consumes xT strips as they arrive
        QKFiller(0).drain()

        # masks / bv ride the sync queue behind wv (needed only once
        # attention starts); the gpsimd queue stays clear for q/k weights
        mk_sb = consts.tile([P, 2, 1024], BF16)
        nc.sync.dma_start(mk_sb[:], mk_d[:])
        bv_sb = consts.tile([P, HL * D], F32)
        nc.sync.dma_start(bv_sb[:], bv_d[:])

        # stage B: v = x @ w_v, natural layout v_sb[p=T within tt, tt, hl*D]
        v_sb = vpool.tile([P, KT, HL * D], BF16)
        for tt in range(KT):
            ps = mm.tile([P, 1024], F32, tag="mm")
            for kt in range(KT):
                for wc in range(2):
                    nc.tensor.matmul(
                        ps[:, wc * 512:(wc + 1) * 512],
                        xt[:, kt, tt * P:(tt + 1) * P],
                        wv[:, kt, wc * 512:(wc + 1) * 512],
                        start=(kt == 0), stop=(kt == KT - 1),
                    )
            nc.vector.tensor_tensor(
                v_sb[:, tt, :], ps[:], bv_sb[:], AluOpType.add,
            )

        yt = mid.tile([P, HL, T], BF16, tag="mid")  # waits until wv is dead

        wp = None
        for h in range(HL):
            qh = qk_tiles[(h, "q")]
            kh = qk_tiles[(h, "k")]
            if h + 1 < HL:
                filler = QKFiller(h + 1)
                steps_total = 4 * KT
            else:
                # xt is dead after head 7's q/k; stream w_proj in and emit the
                # output projection as this head's attention chunks complete
                wp = big.tile([P, HL, C], BF16, tag="big")
                nc.sync.dma_start(wp[:], wp_d[:])
                filler = OutFiller(wp)
                steps_total = 0  # supply added per chunk
            acc = 0.0
            pairs_left = sum(2 * (c + 1) for c in range(TCH))

            for c in range(TCH):
                jmax = 4 * c + 3
                npair = (jmax + 1) // 2
                ps_y = psy.tile([P, 512], F32, tag="y")
                ps_d = psd.tile([P, 512], F32)
                pend = None   # pending pair-sum for the denominator tree
                pend4 = None  # pending 4-sum (tile, contains_jp0)
                for jp in range(npair):
                    diag = 2 * jp >= 4 * c
                    # in diagonal blocks, columns i < 128p of j-tile offset p
                    # are fully causal-masked: narrow S/exp/mask/AV/ones to
                    # the live range instead of computing and zeroing them
                    skips = []
                    ps_s = mm.tile([P, 1024], F32, tag="mm")
                    for half in range(2):
                        jt = 2 * jp + half
                        sk = 128 * (jt - 4 * c) if diag and jt > 4 * c else 0
                        skips.append(sk)
                        nc.tensor.matmul(
                            ps_s[:, half * 512 + sk:(half + 1) * 512],
                            kh[:, jt * P:(jt + 1) * P],
                            qh[:, c * 512 + sk:(c + 1) * 512],
                            start=True, stop=True,
                        )
                    pt = ppool.tile([P, 1024], BF16)
                    if not diag:
                        nc.scalar.activation(pt[:], ps_s[:], EXP)
                    else:
                        p2 = jp - 2 * c
                        for half in range(2):
                            sk = skips[half]
                            sl = slice(half * 512 + sk, (half + 1) * 512)
                            nc.scalar.activation(pt[:, sl], ps_s[:, sl], EXP)
                            nc.vector.tensor_tensor(
                                pt[:, sl], pt[:, sl], mk_sb[:, p2, sl],
                                AluOpType.mult,
                            )
                    for half in range(2):
                        jt = 2 * jp + half
                        sk = skips[half]
                        nc.tensor.matmul(
                            ps_y[:, sk:512], v_sb[:, jt, h * D:(h + 1) * D],
                            pt[:, half * 512 + sk:(half + 1) * 512],
                            start=(jt == 0), stop=(jt == jmax),
                        )
                    if not diag:
                        # column-sum of (p0 + p1) == sum of both column-sums:
                        # pre-sum the pair on DVE; combine two pair-sums into
                        # one ones-matmul (4-way tree, non-diag count is even)
                        pts = spool.tile([P, 512], BF16, tag="pts")
                        nc.vector.tensor_tensor(
                            pts[:], pt[:, 0:512], pt[:, 512:1024], AluOpType.add
                        )
                        if pend is None:
                            pend = (pts, jp)
                        else:
                            pts2 = spool.tile([P, 512], BF16, tag="pts2")
                            nc.vector.tensor_tensor(
                                pts2[:], pend[0][:], pts[:], AluOpType.add
                            )
                            if pend4 is None:
                                pend4 = (pts2, pend[1] == 0)
                            else:
                                pts3 = spool.tile([P, 512], BF16, tag="pts2")
                                nc.vector.tensor_tensor(
                                    pts3[:], pend4[0][:], pts2[:], AluOpType.add
                                )
                                nc.tensor.matmul(
                                    ps_d[:], ones_mat[:], pts3[:],
                                    start=pend4[1], stop=False,
                                )
                                pend4 = None
                            pend = None
                    else:
                        # flush any pending 4-sum BEFORE diagonal ones-matmuls
                        # so the group's start=True matmul executes first
                        if pend4 is not None:
                            nc.tensor.matmul(
                                ps_d[:], ones_mat[:], pend4[0][:],
                                start=pend4[1], stop=False,
                            )
                            pend4 = None
                        for half in range(2):
                            jt = 2 * jp + half
                            sk = skips[half]
                            nc.tensor.matmul(
                                ps_d[:, sk:512], ones_mat[:],
                                pt[:, half * 512 + sk:(half + 1) * 512],
                                start=(jt == 0), stop=(jt == jmax),
                            )
                    # weave in exp-independent projection matmuls
                    if h + 1 < HL:
                        rate = 2.0 if h + 1 == HL - 1 else 1.0
                        acc += rate * steps_total / pairs_left
                        while acc >= 1.0 and filler.step():
                            acc -= 1.0
                    else:
                        for _ in range(3):
                            filler.step()
                assert pend is None and pend4 is None  # flushed at first diag
                # normalization, decoupled from the PE pipeline
                yu = spool.tile([P, 512], F32, tag="yu")
                nc.vector.tensor_copy(yu[:], ps_y[:])
                rc = spool.tile([P, 512], F32, tag="rc")
                nc.vector.reciprocal_approx_fast(rc[:], ps_d[:])
                nc.vector.tensor_tensor(
                    yt[:, h, c * 512:(c + 1) * 512], yu[:], rc[:], AluOpType.mult
                )
                if h + 1 == HL:
                    # rows 4c..4c+3 of the output are now fully determined
                    for tt in range(4 * c, 4 * c + 4):
                        filler.add(tt)
            filler.drain()

    nc.compile()
    return nc


def get_nc():
    if not _NC_CACHE:
        _NC_CACHE.append(build_nc())
    return _NC_CACHE[0]


def _bf(a):
    return np.ascontiguousarray(a).astype(ml_dtypes.bfloat16)


def _shard_inputs(x, w_attn, b_attn, w_proj):
    """Build the 8 per-core in_maps."""
    scale = 1.0 / math.sqrt(D)
    w_q, w_k, w_v = w_attn[:, :C], w_attn[:, C:2 * C], w_attn[:, 2 * C:]
    b_q, b_k, b_v = b_attn[:C], b_attn[C:2 * C], b_attn[2 * C:]

    # causal masks for the 4 j-tile positions within a 512 diagonal block,
    # paired as [2, 128, 1024]: pair 0 = (jt offset 0, 1), pair 1 = (2, 3)
    j = np.arange(P)[:, None]
    i = np.arange(512)[None, :]
    m4 = [(j + P * p <= i).astype(np.float32) for p in range(4)]
    masks = np.stack([np.concatenate([m4[0], m4[1]], axis=1),
                      np.concatenate([m4[2], m4[3]], axis=1)])
    masks_bf = _bf(masks.transpose(1, 0, 2))  # [128, 2, 1024]

    per_hg = {}
    for hg in range(HG):
        s = slice(hg * HL * D, (hg + 1) * HL * D)
        wq = _bf((w_q[:, s] * scale).reshape(KT, P, HL, D).transpose(2, 1, 0, 3))
        wk = _bf(w_k[:, s].reshape(KT, P, HL, D).transpose(2, 1, 0, 3))
        wv = _bf(w_v[:, s].reshape(KT, P, HL * D).transpose(1, 0, 2))
        wp = _bf(w_proj[s, :].reshape(HL, P, C).transpose(1, 0, 2))
        bq = np.ascontiguousarray(
            (b_q[s] * scale).reshape(HL, P).T).astype(np.float32)
        bk = np.ascontiguousarray(b_k[s].reshape(HL, P).T).astype(np.float32)
        bv = np.ascontiguousarray(
            np.broadcast_to(b_v[s], (P, HL * D))).astype(np.float32)
        per_hg[hg] = dict(wq=wq, wk=wk, wv=wv, wp=wp, bq=bq, bk=bk, bv=bv)

    in_maps = []
    for core in range(8):
        b, hg = core // HG, core % HG
        xt = _bf(x[b].T.reshape(KT, P, T).transpose(1, 0, 2))
        in_maps.append({"xt": xt, "masks": masks_bf, **per_hg[hg]})
    return in_maps


def kernel(x, w_attn, b_attn, w_proj, b_proj):
    global LAST_RESULTS
    x = np.asarray(x, dtype=np.float32)
    w_attn = np.asarray(w_attn, dtype=np.float32)
    b_attn = np.asarray(b_attn, dtype=np.float32)
    w_proj = np.asarray(w_proj, dtype=np.float32)
    b_proj = np.asarray(b_proj, dtype=np.float32)

    nc = get_nc()
    in_maps = _shard_inputs(x, w_attn, b_attn, w_proj)
    res = run_bass_kernel_spmd(nc, in_maps, core_ids=list(range(8)))
    LAST_RESULTS = res

    out = np.empty((B, T, C), dtype=np.float32)
    for b in range(B):
        out[b] = res.results[HG * b]["out"] + res.results[HG * b + 1]["out"]
        out[b] += b_proj[None, :]
    return out
